# revision 1
# baseline (speedup 1.0000x reference)
"""Bass/Trainium2 kernel for attention-energy softmax:
  proj = enc @ W.T + b        [S,B,D]
  energies[b,s] = hidden[b] . proj[s,b]
  out = softmax(energies, axis=s)[:, None, :]

Algebraic fusion: energies[b,s] = (hidden[b] @ W) . enc[s,b] + hidden[b].b
The bias term is constant per b and cancels in softmax, so it is dropped.
v = hidden @ W is a tiny [B,D]x[D,D] matmul; the kernel then streams the
512MB encoder_outputs once (memory bound), data-parallel over B on 8 cores.

Per-core pipeline:
  1. W|hidden.T arrives in 8 chunked DMAs; fp32 v matmuls overlap the
     load on a pre-warmed PE.
  2. v is broadcast to all 128 partitions with fp32 selector matmuls
     (no HBM traffic).
  3. 15x 4MB + 2x 2MB fully-contiguous DMAs of enc; each followed by fused
     multiply+reduce (scalar_tensor_tensor with accum) ops on DVE.
  4. Softmax in two overlapped groups with on-chip cross-partition max/sum
     via PE transpose / ones-matmul and diagonal-matmul broadcasts.
"""

import numpy as np

import concourse.bass as bass
import concourse.mybir as mybir
from concourse import bacc
from concourse.masks import make_identity
from concourse.bass_utils import run_bass_kernel_spmd
from concourse.tile import TileContext

S, B, D = 2048, 64, 1024
NCORES = 8
BL = B // NCORES  # 8 local batches per core
P = 128
T = S // P  # 16 seq tiles
EC = D // P  # 8 contraction chunks
F32 = mybir.dt.float32
BF16 = mybir.dt.bfloat16

TRACE = False  # test.py sets True to profile

_CACHE = {}


def _stats_exp(nc, small, pstat, ident, ones8, e_all, g0, gw):
    """Cross-partition max and exp (with accumulated sums) for b in
    [g0, g0+gw). Returns the per-partition sums tile."""
    m8g = small.tile([P, gw], F32, tag=f"m8{g0}")
    nc.vector.tensor_reduce(
        out=m8g,
        in_=e_all[:, g0 : g0 + gw, :],
        axis=mybir.AxisListType.X,
        op=mybir.AluOpType.max,
    )
    trm = pstat.tile([gw, P], F32, tag="stat")
    nc.tensor.transpose(trm, m8g, ident)
    mt = small.tile([gw, P], F32, tag=f"mt{g0}")
    nc.vector.tensor_copy(out=mt, in_=trm)
    gmax = small.tile([gw, 1], F32, tag=f"gmax{g0}")
    nc.vector.tensor_reduce(
        out=gmax, in_=mt, axis=mybir.AxisListType.X, op=mybir.AluOpType.max
    )
    gneg = small.tile([gw, 1], F32, tag=f"gneg{g0}")
    nc.vector.tensor_scalar_mul(gneg, gmax, -1.0)
    diag = small.tile([gw, gw], F32, tag=f"diag{g0}")
    nc.vector.tensor_scalar_mul(diag, ident[0:gw, 0:gw], gneg)
    ngps = pstat.tile([P, gw], F32, tag="stat")
    nc.tensor.matmul(ngps, ones8[0:gw, :], diag, start=True, stop=True)
    negg = small.tile([P, gw], F32, tag=f"negg{g0}")
    nc.vector.tensor_copy(out=negg, in_=ngps)
    s8g = small.tile([P, gw], F32, tag=f"s8{g0}")
    for j in range(gw):
        b = g0 + j
        nc.scalar.activation(
            out=e_all[:, b, :],
            in_=e_all[:, b, :],
            func=mybir.ActivationFunctionType.Exp,
            bias=negg[:, j : j + 1],
            accum_out=s8g[:, j : j + 1],
        )
    return s8g


def _recip_bcast(nc, small, pstat, ones_col, ones_row, s8g, gw):
    """1/sum per b, broadcast to all partitions via K=1 ones-matmul."""
    smps = pstat.tile([1, gw], F32, tag="stat")
    nc.tensor.matmul(smps, ones_col, s8g, start=True, stop=True)
    srow = small.tile([1, gw], F32, tag="srow")
    nc.vector.tensor_copy(out=srow, in_=smps)
    rrow = small.tile([1, gw], F32, tag="rrow")
    nc.vector.reciprocal(rrow, srow)
    rps = pstat.tile([P, gw], F32, tag="stat")
    nc.tensor.matmul(rps, ones_row, rrow, start=True, stop=True)
    recipg = small.tile([P, gw], F32, tag="recip")
    nc.vector.tensor_copy(out=recipg, in_=rps)
    return recipg


def build_kernel() -> bass.Bass:
    nc = bacc.Bacc(None, target_bir_lowering=False)
    enc = nc.dram_tensor("enc", [S, BL, D], F32, kind="ExternalInput")
    wx = nc.dram_tensor("wx", [D, D + BL], F32, kind="ExternalInput")
    out = nc.dram_tensor("out", [BL, S], F32, kind="ExternalOutput")
    DB = D + BL

    with TileContext(nc) as tc:
        with (
            tc.tile_pool(name="consts", bufs=1) as consts,
            tc.tile_pool(name="work", bufs=3) as work,
            tc.tile_pool(name="small", bufs=2) as small,
            tc.tile_pool(name="mm", bufs=2, space="PSUM") as mmp,
            tc.tile_pool(name="ptr", bufs=2, space="PSUM") as ptr,
            tc.tile_pool(name="pstat", bufs=2, space="PSUM") as pstat,
        ):
            ident = consts.tile([P, P], F32)
            make_identity(nc, ident)
            # Warm the PE p-state (needs ~3us of continuous work to reach
            # 2.4GHz) while the weight DMAs are in flight, so the v matmuls
            # run at full clock.
            warm_ps = pstat.tile([P, P], F32, tag="warm")
            for _ in range(8):
                nc.tensor.matmul(warm_ps, ident, ident, start=True, stop=True)

            # ---- chunked load of [W|hT]; v matmuls overlap the DMA ----
            wx_r = wx[:, :].rearrange("(c p) d -> p c d", p=P)
            wx_sb = []
            for c in range(EC):
                wt = consts.tile([P, 1, DB], F32, tag=f"wx{c}")
                nc.sync.dma_start(out=wt, in_=wx_r[:, c : c + 1, :])
                wx_sb.append(wt)

            # selector tiles: sel[k, b, m] = 1 if k == b else 0
            ones8 = consts.tile([BL, P], F32)
            nc.vector.memset(ones8, 1.0)
            sel = consts.tile([BL, BL, P], F32)
            for b in range(BL):
                nc.vector.tensor_scalar_mul(
                    sel[:, b, :], ones8, ident[0:BL, b : b + 1]
                )

            # v = hidden_local @ W -> [BL, D] (all fp32 for accuracy).
            # Both column halves accumulate in parallel PSUM banks, chunk by
            # chunk, so v completes right after the last weight chunk lands.
            v_sb = consts.tile([BL, D], F32)
            v_ps0 = mmp.tile([BL, 512], F32, tag="mm")
            v_ps1 = mmp.tile([BL, 512], F32, tag="mm")
            for c in range(EC):
                for h, v_ps in ((0, v_ps0), (1, v_ps1)):
                    nc.tensor.matmul(
                        v_ps,
                        wx_sb[c][:, 0, D : D + BL],
                        wx_sb[c][:, 0, h * 512 : (h + 1) * 512],
                        start=(c == 0),
                        stop=(c == EC - 1),
                    )
            nc.scalar.copy(out=v_sb[:, 0:512], in_=v_ps0)
            nc.scalar.copy(out=v_sb[:, 512:1024], in_=v_ps1)

            # ---- broadcast v to all partitions: vb[p, b, d] = v[b, d] ----
            vb = consts.tile([P, BL, D], F32)
            for b in range(BL):
                for h in range(2):
                    bc_ps = mmp.tile([P, 512], F32, tag="mm")
                    nc.tensor.matmul(
                        bc_ps,
                        sel[:, b, :],
                        v_sb[:, h * 512 : (h + 1) * 512],
                        start=True,
                        stop=True,
                    )
                    # b=0 copies on the (still idle) vector engine so the
                    # first multiply can start ASAP; the rest stream on ACT
                    eng = nc.vector.tensor_copy if b == 0 else nc.scalar.copy
                    eng(out=vb[:, b, h * 512 : (h + 1) * 512], in_=bc_ps)

            # ---- energies: e_all[p, b, t] = sum_d enc[t*128+p, b, d]*v[b, d] ----
            e_all = consts.tile([P, BL, T], F32)
            dummy = consts.tile([P, 1], F32)
            ones_col = consts.tile([P, 1], F32)
            nc.vector.memset(ones_col, 1.0)
            ones_row = consts.tile([1, P], F32)
            nc.vector.memset(ones_row, 1.0)
            out_r = out[:, :].rearrange("b (t p) -> t b p", p=P)
            out_t = consts.tile([T, BL, P], F32)
            GW = BL // 2  # softmax group width

            def stt(src, j, b, t):
                # fused multiply + free-dim sum in one DVE pass:
                # out = (in0 * 1.0) * in1, accum = sum(out)
                nc.vector.scalar_tensor_tensor(
                    out=dummy.broadcast_to((P, D)),
                    in0=src[:, j, :],
                    scalar=1.0,
                    in1=vb[:, b, :],
                    op0=mybir.AluOpType.mult,
                    op1=mybir.AluOpType.mult,
                    accum_out=e_all[:, b, t : t + 1],
                )

            for t in range(T - 1):
                enc_t = work.tile([P, BL, D], F32, tag="enc_t")
                nc.sync.dma_start(out=enc_t, in_=enc[t * P : (t + 1) * P, :, :])
                for b in range(BL):
                    stt(enc_t, b, b, t)
            # last seq tile split by b-halves so group-0 softmax overlaps
            # the remaining multiply stream
            t = T - 1
            last_halves = []
            for gh in range(2):
                eh = work.tile([P, GW, D], F32, tag="enc_t")
                nc.sync.dma_start(
                    out=eh, in_=enc[t * P : (t + 1) * P, gh * GW : (gh + 1) * GW, :]
                )
                last_halves.append(eh)
            for gh in range(2):
                for j in range(GW):
                    stt(last_halves[gh], j, gh * GW + j, t)
                if gh == 0:
                    s8_0 = _stats_exp(nc, small, pstat, ident, ones8, e_all, 0, GW)
            s8_1 = _stats_exp(nc, small, pstat, ident, ones8, e_all, GW, GW)
            for g0, s8g in ((0, s8_0), (GW, s8_1)):
                recipg = _recip_bcast(
                    nc, small, pstat, ones_col, ones_row, s8g, GW
                )
                for j in range(GW):
                    b = g0 + j
                    scl = small.tile([P, T], F32, tag="scl")
                    nc.vector.tensor_scalar_mul(
                        scl, e_all[:, b, :], recipg[:, j : j + 1]
                    )
                    tr = ptr.tile([T, P], F32, tag="tr")
                    nc.tensor.transpose(tr, scl, ident)
                    nc.scalar.copy(out=out_t[:, b, :], in_=tr)
                nc.sync.dma_start(
                    out=out_r[:, g0 : g0 + GW, :], in_=out_t[:, g0 : g0 + GW, :]
                )

    nc.compile()
    return nc


def kernel(hidden, encoder_outputs, W_attn, b_attn):
    hidden = np.asarray(hidden, dtype=np.float32)
    encoder_outputs = np.asarray(encoder_outputs, dtype=np.float32)
    W_attn = np.asarray(W_attn, dtype=np.float32)

    in_maps = []
    for c in range(NCORES):
        bs = slice(c * BL, (c + 1) * BL)
        in_maps.append(
            {
                "enc": np.ascontiguousarray(encoder_outputs[:, bs, :]),
                "wx": np.ascontiguousarray(
                    np.concatenate([W_attn, hidden[0, bs, :].T], axis=1)
                ),
            }
        )

    if "nc" not in _CACHE:
        _CACHE["nc"] = build_kernel()
    nc = _CACHE["nc"]

    res = run_bass_kernel_spmd(nc, in_maps, core_ids=list(range(NCORES)), trace=TRACE)
    if TRACE:
        _CACHE["last_result"] = res
    out = np.concatenate([r["out"] for r in res.results], axis=0)  # [B, S]
    return out[:, None, :]

